# revision 1
# baseline (speedup 1.0000x reference)
"""Trainium2 Bass kernel for an MQA attention block (8 q-heads, shared K/V).

Sharding: 8 cores; core c -> batch b=c//4, query rows s0=(c%4)*512 .. +512,
all 8 heads.  K/V (full sequence, per batch) are computed redundantly on each
core; there is no cross-core communication.  Host folds the two RMSBatchNorm
evals into the projection weights, and DQ^-0.5 into the q-layernorm affine.

Projections run in feature-on-partitions ("T") layout so every matmul keeps a
512-wide moving dim (full-rate float32r).  LayerNorm + RoPE run in row layout
(positions on partitions) after a PE transpose, so LN stats are per-partition
scalars and per-feature affines are host-replicated constant tiles.  Softmax
needs no max-subtraction (logits softcapped to +-5); the denominator comes
from a ones-column appended to V.
"""

import os
import sys

for _p in ("/opt/trn_rl_repo",):
    if _p not in sys.path and os.path.isdir(_p):
        sys.path.insert(0, _p)

import numpy as np
from contextlib import ExitStack

import concourse.bass as bass
import concourse.mybir as mybir
import concourse.tile as tile
from concourse import bacc
from concourse import bass_utils

F32 = mybir.dt.float32
F32R = mybir.dt.float32r

# problem shapes (hardcoded per contract)
B, S, D = 2, 2048, 1536
H, DQ, DK, DV = 8, 128, 128, 192
P = 128
SQ = S // 4          # 512 query rows per core
DC = D // P          # 12 contraction chunks
JC = S // P          # 16 key chunks
SC = SQ // P         # 4 query-row chunks
NCORES = 8
EPS_RMS = 1e-6
EPS_LN = 1e-5
SOFTCAP = 5.0
ROPE_BASE = 8192.0
VPAD = 256           # v' row width: 192 v + 1 ones + 63 zero (N>=256 for f32r)
HALF = DQ // 2


def _r(ap):
    """bitcast an fp32 AP to float32r for full-rate PE matmuls"""
    return ap.bitcast(F32R)


PHASE_LIMIT = {"kv": 1, "q": 2, "attn": 3, "full": 4}[
    os.environ.get("KERNEL_PHASES", "full")
]
REPEAT = int(os.environ.get("KERNEL_REPEAT", "1"))


def build_program(repeat=None):
    global REPEAT
    if repeat is not None:
        REPEAT = repeat
    nc = bacc.Bacc(
        "TRN2", target_bir_lowering=False, debug=False, num_devices=NCORES
    )

    def din(name, shape):
        return nc.dram_tensor(name, list(shape), F32, kind="ExternalInput").ap()

    # per-core inputs
    xT = din("xT", (D, S))
    xTq = din("xTq", (D, SQ))
    biasT = din("biasT", (S, SQ))
    cosq_t = din("cosq", (SQ, HALF))
    sinq_t = din("sinq", (SQ, HALF))
    # shared (replicated) inputs
    cosk_t = din("cosk", (S, HALF))
    sink_t = din("sink", (S, HALF))
    wq = din("wq", (D, H * DQ))
    wk = din("wk", (D, DK))
    wv = din("wv", (D, DV))
    wo = din("wo", (H * DV, D))
    bq_b = din("bq", (P, H))      # folded rms1 bias through Wq, per (d, h)
    bk_b = din("bk", (P, 1))
    bv_b = din("bv", (DV, 1))
    qgr_t = din("qgr", (P, DQ))   # row-replicated LN affines
    qbr_t = din("qbr", (P, DQ))
    kgr_t = din("kgr", (P, DK))
    kbr_t = din("kbr", (P, DK))
    vgr_t = din("vgr", (P, DV))
    vbr_t = din("vbr", (P, DV))
    bor_t = din("bor", (P, D))    # row-replicated output bias
    vpad_t = din("vpad", (P, JC * (VPAD - DV)))  # ones-col + zero pad for v'
    ident = din("ident", (P, P))
    out = nc.dram_tensor("out", [SQ, D], F32, kind="ExternalOutput").ap()

    TT = mybir.AluOpType
    AF = mybir.ActivationFunctionType
    AX = mybir.AxisListType

    with tile.TileContext(nc) as tc, ExitStack() as ctx:
        const = ctx.enter_context(tc.tile_pool(name="const", bufs=1))
        persist = ctx.enter_context(tc.tile_pool(name="persist", bufs=1))

        # ---- small constants into SBUF (live whole kernel)
        ident_sb = const.tile([P, P], F32)
        nc.sync.dma_start(ident_sb[:], ident)
        bk_sb = const.tile([P, 1], F32)
        nc.sync.dma_start(bk_sb[:], bk_b)
        bvA = const.tile([P, 1], F32)
        nc.sync.dma_start(bvA[:], bv_b[:P, :])
        bvB = const.tile([DV - P, 1], F32)
        nc.sync.dma_start(bvB[:], bv_b[P:, :])
        bqh_sb = const.tile([P, H], F32)
        nc.sync.dma_start(bqh_sb[:], bq_b)
        qgr = const.tile([P, DQ], F32)
        nc.sync.dma_start(qgr[:], qgr_t)
        qbr = const.tile([P, DQ], F32)
        nc.sync.dma_start(qbr[:], qbr_t)
        kgr = const.tile([P, DK], F32)
        nc.sync.dma_start(kgr[:], kgr_t)
        kbr = const.tile([P, DK], F32)
        nc.sync.dma_start(kbr[:], kbr_t)
        vgr = const.tile([P, DV], F32)
        nc.sync.dma_start(vgr[:], vgr_t)
        vbr = const.tile([P, DV], F32)
        nc.sync.dma_start(vbr[:], vbr_t)
        bor = const.tile([P, D], F32)
        nc.sync.dma_start(bor[:], bor_t)
        eps_sb = const.tile([P, 1], F32)
        nc.vector.memset(eps_sb[:], EPS_LN)

        # persistent activation tensors
        kT_sb = persist.tile([P, S], F32)            # rope'd LN'd k, [dk, s]
        vrow_sb = persist.tile([P, JC, VPAD], F32)   # v rows + ones col
        qT_sb = persist.tile([P, H, SQ], F32)        # rope'd LN'd q, [dq,h,i]
        yatt_sb = persist.tile([P, SC, H * DV], F32)  # attn out rows

        nc.sync.dma_start(
            _r(vrow_sb[:, :, DV:]),
            _r(vpad_t.rearrange("p (jc f) -> p jc f", jc=JC)),
        )

        def ln_rows(pool, src_sb, width, inv_n, grep, brep, out_ap, tag):
            """LayerNorm rows of src_sb [P, width] over the free dim, then
            out = norm * grep + brep, written to out_ap ([P, width])."""
            st = pool.tile([P, 4], F32, tag=tag + "st")
            nc.vector.tensor_reduce(
                st[:, 0:1], src_sb[:], axis=AX.X, op=TT.add
            )
            sq = pool.tile([P, width], F32, tag=tag + "sq")
            nc.vector.tensor_tensor(sq[:], src_sb[:], src_sb[:], TT.mult)
            nc.vector.tensor_reduce(
                st[:, 1:2], sq[:], axis=AX.X, op=TT.add
            )
            # [mu, m2] = [sum, sumsq] * inv_n
            nc.vector.tensor_scalar(
                st[:, 0:2], st[:, 0:2], inv_n, None, TT.mult
            )
            nc.vector.tensor_tensor(st[:, 2:3], st[:, 0:1], st[:, 0:1],
                                    TT.mult)
            nc.vector.tensor_tensor(st[:, 3:4], st[:, 1:2], st[:, 2:3],
                                    TT.subtract)
            nc.scalar.activation(
                st[:, 3:4], st[:, 3:4], AF.Sqrt, bias=eps_sb[:, 0:1]
            )
            nc.vector.reciprocal(st[:, 3:4], st[:, 3:4])
            xn = pool.tile([P, width], F32, tag=tag + "xn")
            nc.vector.tensor_scalar(
                xn[:], src_sb[:], st[:, 0:1], st[:, 3:4],
                TT.subtract, TT.mult,
            )
            nc.vector.tensor_tensor(xn[:], xn[:], grep[:], TT.mult)
            nc.vector.tensor_tensor(out_ap, xn[:], brep[:], TT.add)
            return out_ap

        def rope_rows(pool, xn, cos_t, sin_t, out_ap, tag):
            """out[:, :64] = x1*cos - x2*sin ; out[:, 64:] = x1*sin + x2*cos"""
            x1 = xn[:, :HALF]
            x2 = xn[:, HALF:]
            t1 = pool.tile([P, HALF], F32, tag=tag + "t1")
            nc.vector.tensor_tensor(out_ap[:, :HALF], x1, cos_t, TT.mult)
            nc.vector.tensor_tensor(t1[:], x2, sin_t, TT.mult)
            nc.vector.tensor_tensor(
                out_ap[:, :HALF], out_ap[:, :HALF], t1[:], TT.subtract
            )
            nc.vector.tensor_tensor(out_ap[:, HALF:], x1, sin_t, TT.mult)
            nc.vector.tensor_tensor(t1[:], x2, cos_t, TT.mult)
            nc.vector.tensor_tensor(
                out_ap[:, HALF:], out_ap[:, HALF:], t1[:], TT.add
            )

        JH = S // 2  # columns per half

        for _rep in range(REPEAT):
          # ========================================================
          # Phase KV: k/v projections (T layout) + LN/rope (rows)
          # ========================================================

         with (
             tc.tile_pool(name="kvc", bufs=1) as kvc,
             tc.tile_pool(name="kvbig", bufs=1) as kvbig,
             tc.tile_pool(name="kvp", bufs=2) as kvp,
             tc.tile_pool(name="kvpsum", bufs=1, space="PSUM") as kvps,
         ):
             wk_sb = kvc.tile([P, DC, DK], F32)
             nc.sync.dma_start(_r(wk_sb[:]), _r(wk.rearrange("(c p) f -> p c f", p=P)))
             wv_sb = kvc.tile([P, DC, DV], F32)
             nc.sync.dma_start(_r(wv_sb[:]), _r(wv.rearrange("(c p) f -> p c f", p=P)))
             cosk_sb = kvc.tile([P, JC, HALF], F32)
             nc.sync.dma_start(
                 cosk_sb[:], cosk_t.rearrange("(jc p) f -> p jc f", p=P)
             )
             sink_sb = kvc.tile([P, JC, HALF], F32)
             nc.sync.dma_start(
                 sink_sb[:], sink_t.rearrange("(jc p) f -> p jc f", p=P)
             )

             for jh in range(2):
                 j0 = jh * JH
                 kT_ps = kvps.tile([P, JH], F32, tag="kT_ps")
                 vTa_ps = kvps.tile([P, JH], F32, tag="vTa_ps")
                 vTb_ps = kvps.tile([DV - P, JH], F32, tag="vTb_ps")
                 for dc in range(DC):
                     xt = kvp.tile([P, JH], F32, tag="xt")
                     nc.sync.dma_start(
                         _r(xt[:]), _r(xT[dc * P : (dc + 1) * P, j0 : j0 + JH])
                     )
                     for n in range(JH // 512):
                         cs = slice(n * 512, (n + 1) * 512)
                         nc.tensor.matmul(
                             kT_ps[:, cs],
                             _r(wk_sb[:, dc, :]),
                             _r(xt[:, cs]),
                             start=(dc == 0),
                             stop=(dc == DC - 1),
                         )
                         nc.tensor.matmul(
                             vTa_ps[:, cs],
                             _r(wv_sb[:, dc, :P]),
                             _r(xt[:, cs]),
                             start=(dc == 0),
                             stop=(dc == DC - 1),
                         )
                         nc.tensor.matmul(
                             vTb_ps[:, cs],
                             _r(wv_sb[:, dc, P:]),
                             _r(xt[:, cs]),
                             start=(dc == 0),
                             stop=(dc == DC - 1),
                         )

                 # drain to SBUF with folded rms1 bias (per-partition in T)
                 k_sb = kvbig.tile([P, JH], F32, tag="k_sb")
                 nc.vector.tensor_scalar_add(k_sb[:], kT_ps[:], bk_sb[:, 0:1])
                 va_sb = kvbig.tile([P, JH], F32, tag="va_sb")
                 nc.vector.tensor_scalar_add(va_sb[:], vTa_ps[:], bvA[:, 0:1])
                 vb_sb2 = kvbig.tile([DV - P, JH], F32, tag="vb_sb2")
                 nc.vector.tensor_scalar_add(
                     vb_sb2[:], vTb_ps[:], bvB[:, 0:1]
                 )

                 for t in range(8):
                     jc = jh * 8 + t
                     tsl = slice(t * P, (t + 1) * P)
                     # ---- k chunk: transpose -> rows
                     scr = kvps.tile([P, 512], F32, tag="scratch", name="scrk")
                     nc.tensor.transpose(scr[:, :P], k_sb[:, tsl], ident_sb[:])
                     krow = kvp.tile([P, P], F32, tag="krow")
                     nc.vector.tensor_copy(krow[:], scr[:, :P])
                     kn = kvp.tile([P, P], F32, tag="kn")
                     ln_rows(kvp, krow, DK, 1.0 / DK, kgr, kbr, kn[:], "k")
                     kr = kvp.tile([P, P], F32, tag="kr")
                     rope_rows(
                         kvp, kn, cosk_sb[:, jc, :], sink_sb[:, jc, :],
                         kr, "k",
                     )
                     # transpose back -> kT
                     scr2 = kvps.tile([P, 512], F32, tag="scratch",
                                      name="scrk2")
                     nc.tensor.transpose(scr2[:, :P], kr[:], ident_sb[:])
                     nc.vector.tensor_copy(
                         _r(kT_sb[:, jc * P : (jc + 1) * P]), scr2[:, :P]
                     )

                     # ---- v chunk: transpose a|b into one row tile
                     scr3 = kvps.tile([P, 512], F32, tag="scratch",
                                      name="scrv")
                     nc.tensor.transpose(
                         scr3[:, :P], va_sb[:, tsl], ident_sb[:]
                     )
                     nc.tensor.transpose(
                         scr3[:, P:DV], vb_sb2[:, tsl],
                         ident_sb[: DV - P, : DV - P],
                     )
                     vrow = kvp.tile([P, DV], F32, tag="vrow")
                     nc.vector.tensor_copy(vrow[:], scr3[:, :DV])
                     ln_rows(
                         kvp, vrow, DV, 1.0 / DV, vgr, vbr,
                         _r(vrow_sb[:, jc, :DV]), "v",
                     )

         # =========================================================
         # Phase Q: per-head q projection (T) + LN/rope (rows)
         # =========================================================
         if PHASE_LIMIT >= 2:
          with (
             tc.tile_pool(name="qc", bufs=1) as qc,
             tc.tile_pool(name="qw", bufs=2) as qw,
             tc.tile_pool(name="qp", bufs=2) as qp,
             tc.tile_pool(name="qpsum", bufs=2, space="PSUM") as qps,
         ):
             xtq_sb = qc.tile([P, DC, SQ], F32)
             nc.sync.dma_start(
                 _r(xtq_sb[:]), _r(xTq.rearrange("(c p) s -> p c s", p=P))
             )
             cosq_sb = qc.tile([P, SC, HALF], F32)
             nc.sync.dma_start(
                 cosq_sb[:], cosq_t.rearrange("(sc p) f -> p sc f", p=P)
             )
             sinq_sb = qc.tile([P, SC, HALF], F32)
             nc.sync.dma_start(
                 sinq_sb[:], sinq_t.rearrange("(sc p) f -> p sc f", p=P)
             )

             wq3 = wq.rearrange("(c p) f -> p c f", p=P)
             for h in range(H):
                 wqh = qw.tile([P, DC, DQ], F32, tag="wqh")
                 nc.sync.dma_start(
                     _r(wqh[:]), _r(wq3[:, :, h * DQ : (h + 1) * DQ])
                 )
                 q_ps = qps.tile([P, SQ], F32, tag="q_ps")
                 for dc in range(DC):
                     nc.tensor.matmul(
                         q_ps[:],
                         _r(wqh[:, dc, :]),
                         _r(xtq_sb[:, dc, :]),
                         start=(dc == 0),
                         stop=(dc == DC - 1),
                     )
                 q_sb = qp.tile([P, SQ], F32, tag="q_sb")
                 nc.vector.tensor_scalar_add(
                     q_sb[:], q_ps[:], bqh_sb[:, h : h + 1]
                 )
                 for sc in range(SC):
                     ssl = slice(sc * P, (sc + 1) * P)
                     scr = qps.tile([P, 512], F32, tag="qscr", name="qscr")
                     nc.tensor.transpose(
                         scr[:, :P], q_sb[:, ssl], ident_sb[:]
                     )
                     qrow = qp.tile([P, P], F32, tag="qrow")
                     nc.vector.tensor_copy(qrow[:], scr[:, :P])
                     qn = qp.tile([P, P], F32, tag="qn")
                     ln_rows(qp, qrow, DQ, 1.0 / DQ, qgr, qbr, qn[:], "q")
                     qr = qp.tile([P, P], F32, tag="qr")
                     rope_rows(
                         qp, qn, cosq_sb[:, sc, :], sinq_sb[:, sc, :],
                         qr, "q",
                     )
                     scr2 = qps.tile([P, 512], F32, tag="qscr", name="qscr2")
                     nc.tensor.transpose(scr2[:, :P], qr[:], ident_sb[:])
                     nc.vector.tensor_copy(
                         _r(qT_sb[:, h, sc * P : (sc + 1) * P]), scr2[:, :P]
                     )

         # =========================================================
         # Attention phase (per head, groups of 2 key chunks)
         # =========================================================
         if PHASE_LIMIT >= 3:
          with tc.tile_pool(name="wop", bufs=1) as wop:
             # prefetch wo during attention
             wo_sb = wop.tile([P, DC, D], F32)
             nc.sync.dma_start(_r(wo_sb[:]), _r(wo.rearrange("(c p) f -> p c f", p=P)))

             with (
                 tc.tile_pool(name="att", bufs=3) as att,
                 tc.tile_pool(name="attb", bufs=1) as attb,
                 tc.tile_pool(name="apsum", bufs=1, space="PSUM") as aps,
                 tc.tile_pool(name="ypsum", bufs=1, space="PSUM") as yps,
             ):
                 biasT_sb = attb.tile([P, JC, SQ], F32)
                 nc.sync.dma_start(
                     biasT_sb[:], biasT.rearrange("(jc p) i -> p jc i", p=P)
                 )

                 y_ps = [
                     yps.tile([P, VPAD], F32, tag=f"y{ic}", name=f"y{ic}")
                     for ic in range(SC)
                 ]
                 for h in range(H):
                     for jg in range(JC // 2):
                         tag = "pq" + str(jg % 2)
                         pq = aps.tile([P, 2, 512], F32, tag=tag, name="pq")
                         for c in range(2):
                             jc = jg * 2 + c
                             nc.tensor.matmul(
                                 pq[:, c, :],
                                 _r(kT_sb[:, jc * P : (jc + 1) * P]),
                                 _r(qT_sb[:, h, :]),
                                 start=True, stop=True,
                             )
                         nc.vector.tensor_tensor(
                             pq[:], pq[:],
                             biasT_sb[:, jg * 2 : jg * 2 + 2, :], TT.add,
                         )
                         nc.scalar.activation(
                             pq[:], pq[:], AF.Tanh, scale=1.0 / SOFTCAP
                         )
                         pt = att.tile([P, 2, 512], F32, tag="pt")
                         nc.scalar.activation(
                             _r(pt[:]), pq[:], AF.Exp, scale=SOFTCAP
                         )
                         for c in range(2):
                             jc = jg * 2 + c
                             for ic in range(SC):
                                 nc.tensor.matmul(
                                     y_ps[ic][:],
                                     _r(pt[:, c, ic * P : (ic + 1) * P]),
                                     _r(vrow_sb[:, jc, :]),
                                     start=(jc == 0),
                                     stop=(jc == JC - 1),
                                 )
                     # drain: normalize rows by the ones-column sum
                     for ic in range(SC):
                         recip = att.tile([P, 1], F32, tag="recip")
                         nc.vector.reciprocal(
                             recip[:], y_ps[ic][:, DV : DV + 1]
                         )
                         nc.vector.tensor_scalar(
                             yatt_sb[:, ic, h * DV : (h + 1) * DV],
                             y_ps[ic][:, :DV],
                             recip[:, 0:1], None, TT.mult,
                         )

             # =====================================================
             # Output projection: transpose y_att, then matmul + bias
             # =====================================================
             if PHASE_LIMIT >= 4:
              with (
                 tc.tile_pool(name="op", bufs=2) as op,
                 tc.tile_pool(name="oyT", bufs=1) as oyT,
                 tc.tile_pool(name="opsum", bufs=2, space="PSUM") as ops,
                 tc.tile_pool(name="otps", bufs=2, space="PSUM") as otps,
             ):
                 yT_sb = oyT.tile([P, DC, SQ], F32, tag="yT")
                 for sc in range(SC):
                     for fc in range(DC):
                         pt2 = otps.tile([P, P], F32, tag="yt")
                         nc.tensor.transpose(
                             pt2[:],
                             yatt_sb[:, sc, fc * P : (fc + 1) * P],
                             ident_sb[:],
                         )
                         nc.vector.tensor_copy(
                             _r(yT_sb[:, fc, sc * P : (sc + 1) * P]), pt2[:]
                         )
                 for sc in range(SC):
                     o_ps = ops.tile([P, D], F32, tag="o_ps")
                     for fc in range(DC):
                         for n in range(D // 512):
                             nc.tensor.matmul(
                                 o_ps[:, n * 512 : (n + 1) * 512],
                                 _r(yT_sb[:, fc, sc * P : (sc + 1) * P]),
                                 _r(wo_sb[:, fc, n * 512 : (n + 1) * 512]),
                                 start=(fc == 0),
                                 stop=(fc == DC - 1),
                             )
                     o_sb = op.tile([P, D], F32, tag="o_sb")
                     nc.vector.tensor_tensor(
                         o_sb[:], o_ps[:], bor[:], TT.add
                     )
                     nc.sync.dma_start(
                         out[sc * P : (sc + 1) * P, :], o_sb[:]
                     )

         if PHASE_LIMIT < 4:
             _finish_debug(nc, tc, out)

    nc.compile()
    return nc


def _finish_debug(nc, tc, out):
    F = mybir.dt.float32
    with tc.tile_pool(name="dbg", bufs=1) as dbg:
        z = dbg.tile([P, D], F)
        nc.vector.memset(z[:], 0.0)
        for sc in range(SC):
            nc.sync.dma_start(out[sc * P : (sc + 1) * P, :], z[:])


def _host_prep(inputs):
    f32 = np.float32
    x = np.asarray(inputs["x"], f32)
    bias = np.asarray(inputs["attention_bias"], f32)
    g1 = np.asarray(inputs["g1"], f32)
    b1 = np.asarray(inputs["b1"], f32)
    rr1 = np.asarray(inputs["rrms1"], f32)
    Wq = np.asarray(inputs["Wq"], f32)
    Wk = np.asarray(inputs["Wk"], f32)
    Wv = np.asarray(inputs["Wv"], f32)
    qg = np.asarray(inputs["qg"], f32)
    qb = np.asarray(inputs["qb"], f32)
    kg = np.asarray(inputs["kg"], f32)
    kb = np.asarray(inputs["kb"], f32)
    vg = np.asarray(inputs["vg"], f32)
    vb = np.asarray(inputs["vb"], f32)
    Wo = np.asarray(inputs["Wo"], f32)
    bo = np.asarray(inputs["bo"], f32)
    g2 = np.asarray(inputs["g2"], f32)
    b2 = np.asarray(inputs["b2"], f32)
    rr2 = np.asarray(inputs["rrms2"], f32)

    scale1 = (g1 * (1.0 / np.sqrt(rr1 + EPS_RMS))).astype(f32)
    Wq_e = (Wq * scale1[:, None]).astype(f32)
    Wk_e = (Wk * scale1[:, None]).astype(f32)
    Wv_e = (Wv * scale1[:, None]).astype(f32)
    bq_row = (b1 @ Wq).astype(f32)      # [H*DQ]
    bk_row = (b1 @ Wk).astype(f32)      # [DK]
    bv_row = (b1 @ Wv).astype(f32)      # [DV]
    sc_q = f32(DQ) ** f32(-0.5)
    qg_e = (qg * sc_q).astype(f32)
    qb_e = (qb * sc_q).astype(f32)
    scale2 = (g2 * (1.0 / np.sqrt(rr2 + EPS_RMS))).astype(f32)
    Wo_e = (Wo * scale2[None, :]).astype(f32)
    bo_e = (bo * scale2 + b2).astype(f32)

    freqs = (
        1.0 / (ROPE_BASE ** (np.arange(HALF, dtype=f32) / HALF))
    ).astype(f32)
    ang = np.arange(S, dtype=f32)[:, None] * freqs[None, :]
    cos = np.cos(ang).astype(f32)                        # [S, 64]
    sin = np.sin(ang).astype(f32)

    rep = lambda v: np.broadcast_to(v[None, :], (P, v.shape[0]))
    shared = {
        "cosk": cos,
        "sink": sin,
        "wq": Wq_e,
        "wk": Wk_e,
        "wv": Wv_e,
        "wo": Wo_e,
        "bq": bq_row.reshape(H, DQ).T,
        "bk": bk_row.reshape(DK, 1),
        "bv": bv_row.reshape(DV, 1),
        "qgr": rep(qg_e),
        "qbr": rep(qb_e),
        "kgr": rep(kg),
        "kbr": rep(kb),
        "vgr": rep(vg),
        "vbr": rep(vb),
        "bor": rep(bo_e),
        "vpad": np.tile(
            np.concatenate(
                [np.ones((P, 1), f32), np.zeros((P, VPAD - DV - 1), f32)],
                axis=1,
            ),
            (1, JC),
        ),
        "ident": np.eye(P, dtype=f32),
    }
    shared = {k: np.ascontiguousarray(v, dtype=f32) for k, v in shared.items()}

    xTs = [np.ascontiguousarray(x[b].T) for b in range(B)]
    in_maps = []
    for c in range(NCORES):
        b = c // 4
        s0 = (c % 4) * SQ
        m = dict(shared)
        m["xT"] = xTs[b]
        m["xTq"] = np.ascontiguousarray(xTs[b][:, s0 : s0 + SQ])
        m["biasT"] = np.ascontiguousarray(bias[0, 0, s0 : s0 + SQ, :].T)
        m["cosq"] = np.ascontiguousarray(cos[s0 : s0 + SQ, :])
        m["sinq"] = np.ascontiguousarray(sin[s0 : s0 + SQ, :])
        in_maps.append(m)
    return in_maps


_NC_CACHE = None


def _get_nc():
    global _NC_CACHE
    if _NC_CACHE is None:
        _NC_CACHE = build_program()
    return _NC_CACHE


def kernel(**inputs) -> np.ndarray:
    nc = _get_nc()
    in_maps = _host_prep(inputs)
    res = bass_utils.run_bass_kernel_spmd(
        nc, in_maps, core_ids=list(range(NCORES))
    )
    outs = res.results
    full = np.empty((B, S, D), np.float32)
    for c in range(NCORES):
        b = c // 4
        s0 = (c % 4) * SQ
        full[b, s0 : s0 + SQ, :] = outs[c]["out"]
    return full


if __name__ == "__main__":
    nc = _get_nc()
    print("build + compile OK")



# revision 6
# speedup vs baseline: 1.9620x; 1.9620x over previous
"""Trainium2 Bass kernel for an MQA attention block (8 q-heads, shared K/V).

Sharding: 8 cores; core c -> batch b=c//4, query rows s0=(c%4)*512 .. +512,
all 8 heads.  K/V (full sequence, per batch) are computed redundantly on each
core; no cross-core communication.

v2 design (bf16 compute, fp32 PSUM):
- Projections in ROW layout: stationary = xT chunk, moving = W.  K|V fused
  into one 320-wide matmul; Q done on the same stationary for its 4 chunks.
- LayerNorm mean-removal is folded into the weights host-side (each output
  block of W is column-centered, so projections have exact zero block-mean).
  Only sum-of-squares is needed on device.
- LN gain + RoPE are folded into host-precomputed multiplier tiles P1/P2/B:
  out = rstd * (y1*P1 + y2*P2) + B  (4 DVE ops per chunk, no transposed LN).
- Softmax denominator via a ones-column appended to V (no max subtraction
  needed: logits are softcapped to +-5).
- Attention works per head-pair: logits 2 heads x 512 q -> [128,1024] tiles
  for bias-add (DVE), tanh + exp (ACT).  attn*V per head with pt stationary.
- y rows -> yT via DMA xbar transposes (overlapped with attention compute).
- Output projection from yT with Wo moving; rms2 folded into Wo host-side;
  v-LN bias folded into the output bias (softmax weights sum to 1).
"""

import os
import sys

for _p in ("/opt/trn_rl_repo",):
    if _p not in sys.path and os.path.isdir(_p):
        sys.path.insert(0, _p)

import numpy as np
import ml_dtypes
from contextlib import ExitStack

import concourse.bass as bass
import concourse.mybir as mybir
import concourse.tile as tile
from concourse import bacc
from concourse import bass_utils

F32 = mybir.dt.float32
BF16 = mybir.dt.bfloat16

# problem shapes (hardcoded per contract)
B, S, D = 2, 2048, 1536
H, DQ, DK, DV = 8, 128, 128, 192
P = 128
SQ = S // 4          # 512 query rows per core
DC = D // P          # 12 contraction chunks
JC = S // P          # 16 key/position chunks
QC = SQ // P         # 4 query-row chunks
NCORES = 8
EPS_RMS = 1e-6
EPS_LN = 1e-5
SOFTCAP = 5.0
ROPE_BASE = 8192.0
HALF = DQ // 2       # 64
VP = 194             # v row width: 192 v + 1 ones + 1 pad
HDV = H * DV         # 1536
FC = HDV // P        # 12 output-contraction chunks

TT = mybir.AluOpType
AF = mybir.ActivationFunctionType
AX = mybir.AxisListType


def build_program():
    nc = bacc.Bacc(
        "TRN2", target_bir_lowering=False, debug=False, num_devices=NCORES
    )

    def din(name, shape, dt=BF16):
        return nc.dram_tensor(name, list(shape), dt, kind="ExternalInput").ap()

    # per-core inputs
    xT = din("xT", (D, S))                  # this core's batch, feature-major
    biasT = din("biasT", (S, SQ))           # attention bias, [key, query]
    p1q = din("p1q", (SQ, H * DQ))          # rope/LN fold tiles for q
    p2q = din("p2q", (SQ, H * DQ))
    bbq = din("bbq", (SQ, H * DQ))
    # shared (replicated) inputs
    wkv = din("wkv", (D, DK + DV))
    wq = din("wq", (D, H * DQ))
    wo = din("wo", (HDV, D))
    p1k = din("p1k", (S, DK))
    p2k = din("p2k", (S, DK))
    bbk = din("bbk", (S, DK))
    bkv_r = din("bkv", (P, DK + DV))        # row-replicated proj bias k|v
    bq_r = din("bqr", (P, H * DQ))          # row-replicated proj bias q
    vg_r = din("vgr", (P, DV))              # row-replicated v LN gain
    bor = din("bor", (P, D), F32)           # row-replicated output bias (f32)
    ident = din("ident", (P, P))
    out = nc.dram_tensor("out", [SQ, D], F32, kind="ExternalOutput").ap()

    with tile.TileContext(nc) as tc, ExitStack() as ctx:
        const = ctx.enter_context(tc.tile_pool(name="const", bufs=1))
        persist = ctx.enter_context(tc.tile_pool(name="persist", bufs=1))

        ident_sb = const.tile([P, P], BF16)
        nc.sync.dma_start(ident_sb[:], ident)
        bkv_sb = const.tile([P, DK + DV], BF16)
        nc.sync.dma_start(bkv_sb[:], bkv_r)
        bq_sb = const.tile([P, H * DQ], BF16)
        nc.sync.dma_start(bq_sb[:], bq_r)
        vg_sb = const.tile([P, DV], BF16)
        nc.sync.dma_start(vg_sb[:], vg_r)
        bor_sb = const.tile([P, D], F32)
        nc.sync.dma_start(bor_sb[:], bor)
        eps_sb = const.tile([P, 1], F32)
        nc.vector.memset(eps_sb[:], EPS_LN)

        # persistent activations
        ky = persist.tile([P, JC, DK], BF16)      # raw centered k rows
        vy = persist.tile([P, JC, DV], BF16)      # raw centered v rows
        qy = persist.tile([P, QC, H * DQ], BF16)  # raw centered q rows
        kT_sb = persist.tile([P, JC, P], BF16)    # final k, feature-major
        qT_sb = persist.tile([P, H, SQ], BF16)    # final q, feature-major
        v_sb = persist.tile([P, JC, VP], BF16)    # final v rows + ones col
        yT_sb = persist.tile([P, FC, SQ], BF16)   # attn out, feature-major
        rk = persist.tile([P, JC], F32)           # 1/std for k chunks
        rv = persist.tile([P, JC], F32)
        rq = persist.tile([P, QC * H], F32)

        # ones column + pad for v (memset once; LN writes only [:, :192])
        nc.vector.memset(v_sb[:, :, DV:], 0.0)
        for jc in range(JC):
            nc.vector.memset(v_sb[:, jc, DV : DV + 1], 1.0)

        # =========================================================
        # Phase A1: projections (row layout), drains with bias
        # =========================================================
        with (
            tc.tile_pool(name="a1w", bufs=1) as a1w,
            tc.tile_pool(name="a1x", bufs=3) as a1x,
            tc.tile_pool(name="a1ps", bufs=2, space="PSUM") as a1ps,
        ):
            wkv_sb = a1w.tile([P, DC, DK + DV], BF16)
            nc.sync.dma_start(wkv_sb[:], wkv.rearrange("(c p) f -> p c f", p=P))
            wq_sb = a1w.tile([P, DC, H * DQ], BF16)
            nc.sync.dma_start(wq_sb[:], wq.rearrange("(c p) f -> p c f", p=P))

            xT4 = xT.rearrange("(c p) (jc i) -> p c jc i", p=P, i=P)
            for jc in range(JC):
                xt = a1x.tile([P, DC, P], BF16, tag="xt")
                nc.sync.dma_start(xt[:], xT4[:, :, jc, :])
                kv_ps = a1ps.tile([P, DK + DV], F32, tag="kv")
                # The host rotates the sequence so this core's q rows are the
                # LAST 4 chunks (SPMD: chunk indices are baked into the
                # program; q last also means kv matmuls only wait on the
                # small wkv DMA at startup).
                qic = jc - (JC - QC) if jc >= JC - QC else None
                q_ps = None
                if qic is not None:
                    q_ps = a1ps.tile([P, H * DQ], F32, tag="q")
                for dc in range(DC):
                    nc.tensor.matmul(
                        kv_ps[:],
                        xt[:, dc, :],
                        wkv_sb[:, dc, :],
                        start=(dc == 0),
                        stop=(dc == DC - 1),
                    )
                    if qic is not None:
                        for n in range(2):
                            nc.tensor.matmul(
                                q_ps[:, n * 512 : (n + 1) * 512],
                                xt[:, dc, :],
                                wq_sb[:, dc, n * 512 : (n + 1) * 512],
                                start=(dc == 0),
                                stop=(dc == DC - 1),
                            )
                # drains (bias add, cast to bf16)
                nc.vector.tensor_tensor(
                    ky[:, jc, :], kv_ps[:, :DK], bkv_sb[:, :DK], TT.add
                )
                nc.vector.tensor_tensor(
                    vy[:, jc, :], kv_ps[:, DK:], bkv_sb[:, DK:], TT.add
                )
                if qic is not None:
                    nc.vector.tensor_tensor(
                        qy[:, qic, :], q_ps[:], bq_sb[:], TT.add
                    )

        # =========================================================
        # Phase A2: batched sum-of-squares -> 1/std
        # =========================================================
        with tc.tile_pool(name="a2", bufs=1) as a2:
            ksq = a2.tile([P, JC, DK], BF16)
            nc.scalar.activation(
                ksq[:].rearrange("p a b -> p (a b)"),
                ky[:].rearrange("p a b -> p (a b)"),
                AF.Square,
            )
            nc.vector.tensor_reduce(rk[:], ksq[:], axis=AX.X, op=TT.add)
            vsq = a2.tile([P, JC, DV], BF16)
            nc.scalar.activation(
                vsq[:].rearrange("p a b -> p (a b)"),
                vy[:].rearrange("p a b -> p (a b)"),
                AF.Square,
            )
            nc.vector.tensor_reduce(rv[:], vsq[:], axis=AX.X, op=TT.add)
            qsq = a2.tile([P, QC, H * DQ], BF16)
            nc.scalar.activation(
                qsq[:].rearrange("p a b -> p (a b)"),
                qy[:].rearrange("p a b -> p (a b)"),
                AF.Square,
            )
            nc.vector.tensor_reduce(
                rq[:],
                qsq[:].rearrange("p a (h f) -> p (a h) f", f=DQ),
                axis=AX.X,
                op=TT.add,
            )
            # rstd = 1/sqrt(ss/n + eps)
            std_t = a2.tile([P, 2 * JC + QC * H], F32)
            nc.scalar.activation(
                std_t[:, :JC], rk[:], AF.Sqrt, bias=eps_sb[:, 0:1], scale=1.0 / DK
            )
            nc.vector.reciprocal(rk[:], std_t[:, :JC])
            nc.scalar.activation(
                std_t[:, JC : 2 * JC], rv[:], AF.Sqrt,
                bias=eps_sb[:, 0:1], scale=1.0 / DV,
            )
            nc.vector.reciprocal(rv[:], std_t[:, JC : 2 * JC])
            nc.scalar.activation(
                std_t[:, 2 * JC :], rq[:], AF.Sqrt,
                bias=eps_sb[:, 0:1], scale=1.0 / DQ,
            )
            nc.vector.reciprocal(rq[:], std_t[:, 2 * JC :])

        # =========================================================
        # Phase A3: LN-apply + RoPE (folded tiles), transposes
        # =========================================================
        with (
            tc.tile_pool(name="a3c", bufs=1) as a3c,
            tc.tile_pool(name="a3", bufs=3) as a3,
            tc.tile_pool(name="a3ps", bufs=2, space="PSUM") as a3ps,
        ):
            p1k_sb = a3c.tile([P, JC, DK], BF16)
            nc.sync.dma_start(p1k_sb[:], p1k.rearrange("(c p) f -> p c f", p=P))
            p2k_sb = a3c.tile([P, JC, DK], BF16)
            nc.sync.dma_start(p2k_sb[:], p2k.rearrange("(c p) f -> p c f", p=P))
            bbk_sb = a3c.tile([P, JC, DK], BF16)
            nc.sync.dma_start(bbk_sb[:], bbk.rearrange("(c p) f -> p c f", p=P))
            p1q_sb = a3c.tile([P, QC, H * DQ], BF16)
            nc.sync.dma_start(p1q_sb[:], p1q.rearrange("(c p) f -> p c f", p=P))
            p2q_sb = a3c.tile([P, QC, H * DQ], BF16)
            nc.sync.dma_start(p2q_sb[:], p2q.rearrange("(c p) f -> p c f", p=P))
            bbq_sb = a3c.tile([P, QC, H * DQ], BF16)
            nc.sync.dma_start(bbq_sb[:], bbq.rearrange("(c p) f -> p c f", p=P))

            for jc in range(JC):
                # ---- k: out = rstd*(y1*P1 + y2*P2) + B, then transpose
                u = a3.tile([P, DK], BF16, tag="ku")
                y1b = ky[:, jc, 0:HALF].unsqueeze(1).broadcast_to([P, 2, HALF])
                y2b = ky[:, jc, HALF:DK].unsqueeze(1).broadcast_to([P, 2, HALF])
                nc.vector.tensor_tensor(
                    u[:].rearrange("p (t f) -> p t f", t=2),
                    p1k_sb[:, jc, :].rearrange("p (t f) -> p t f", t=2),
                    y1b,
                    TT.mult,
                )
                u2 = a3.tile([P, DK], BF16, tag="ku2")
                nc.vector.tensor_tensor(
                    u2[:].rearrange("p (t f) -> p t f", t=2),
                    p2k_sb[:, jc, :].rearrange("p (t f) -> p t f", t=2),
                    y2b,
                    TT.mult,
                )
                nc.vector.tensor_tensor(u[:], u[:], u2[:], TT.add)
                kr = a3.tile([P, DK], BF16, tag="kr")
                nc.vector.scalar_tensor_tensor(
                    out=kr[:], in0=u[:], scalar=rk[:, jc : jc + 1],
                    in1=bbk_sb[:, jc, :], op0=TT.mult, op1=TT.add,
                )
                tp = a3ps.tile([P, P], BF16, tag="tp")
                nc.tensor.transpose(tp[:], kr[:], ident_sb[:])
                nc.scalar.copy(kT_sb[:, jc, :], tp[:])
                # ---- v: out = (y*rstd)*vg  (LN bias folded into bor)
                nc.vector.scalar_tensor_tensor(
                    out=v_sb[:, jc, :DV], in0=vy[:, jc, :],
                    scalar=rv[:, jc : jc + 1], in1=vg_sb[:],
                    op0=TT.mult, op1=TT.mult,
                )

            for qic in range(QC):
                uq = a3.tile([P, H * DQ], BF16, tag="qu")
                y1b = (
                    qy[:, qic, :]
                    .rearrange("p (h t f) -> p h t f", t=2, f=HALF)[:, :, 0:1, :]
                    .broadcast_to([P, H, 2, HALF])
                )
                y2b = (
                    qy[:, qic, :]
                    .rearrange("p (h t f) -> p h t f", t=2, f=HALF)[:, :, 1:2, :]
                    .broadcast_to([P, H, 2, HALF])
                )
                nc.vector.tensor_tensor(
                    uq[:].rearrange("p (h t f) -> p h t f", t=2, f=HALF),
                    p1q_sb[:, qic, :].rearrange("p (h t f) -> p h t f", t=2, f=HALF),
                    y1b,
                    TT.mult,
                )
                uq2 = a3.tile([P, H * DQ], BF16, tag="qu2")
                nc.vector.tensor_tensor(
                    uq2[:].rearrange("p (h t f) -> p h t f", t=2, f=HALF),
                    p2q_sb[:, qic, :].rearrange("p (h t f) -> p h t f", t=2, f=HALF),
                    y2b,
                    TT.mult,
                )
                nc.vector.tensor_tensor(uq[:], uq[:], uq2[:], TT.add)
                # * rstd (per head, broadcast over features)
                rqb = (
                    rq[:, qic * H : (qic + 1) * H]
                    .unsqueeze(2)
                    .broadcast_to([P, H, DQ])
                )
                nc.vector.tensor_tensor(
                    uq[:].rearrange("p (h f) -> p h f", f=DQ),
                    uq[:].rearrange("p (h f) -> p h f", f=DQ),
                    rqb,
                    TT.mult,
                )
                qr = a3.tile([P, H * DQ], BF16, tag="qr")
                nc.vector.tensor_tensor(qr[:], uq[:], bbq_sb[:, qic, :], TT.add)
                for h in range(H):
                    tp = a3ps.tile([P, P], BF16, tag="tp")
                    nc.tensor.transpose(
                        tp[:], qr[:, h * DQ : (h + 1) * DQ], ident_sb[:]
                    )
                    nc.scalar.copy(
                        qT_sb[:, h, qic * P : (qic + 1) * P], tp[:]
                    )

        # =========================================================
        # Attention (per head pair) + yT via DMA transposes
        # =========================================================
        with (
            tc.tile_pool(name="attc", bufs=1) as attc,
            tc.tile_pool(name="att", bufs=3) as att,
            tc.tile_pool(name="attw", bufs=1) as attw,
            tc.tile_pool(name="attps", bufs=2, space="PSUM") as attps,
            tc.tile_pool(name="yps", bufs=1, space="PSUM") as yps,
        ):
            biasT_sb = attc.tile([P, JC, SQ], BF16)
            nc.sync.dma_start(
                biasT_sb[:], biasT.rearrange("(jc p) i -> p jc i", p=P)
            )
            # prefetch wo for phase C
            wo_sb = attw.tile([P, FC, D], BF16)
            nc.sync.dma_start(wo_sb[:], wo.rearrange("(c p) f -> p c f", p=P))
            pt_sb = attc.tile([P, JC, 2, SQ], BF16)

            y_ps = [
                yps.tile([P, VP], F32, tag=f"y{ic}", name=f"y{ic}")
                for ic in range(QC)
            ]
            yrow = attc.tile([P, QC, 2 * DV], BF16)

            def attnv(hh, jc):
                for ic in range(QC):
                    nc.tensor.matmul(
                        y_ps[ic][:],
                        pt_sb[:, jc, hh, ic * P : (ic + 1) * P],
                        v_sb[:, jc, :],
                        start=(jc == 0),
                        stop=(jc == JC - 1),
                    )

            def drain_y(hh):
                for ic in range(QC):
                    recip = att.tile([P, 1], F32, tag="recip")
                    nc.vector.reciprocal(recip[:], y_ps[ic][:, DV : DV + 1])
                    nc.vector.tensor_scalar(
                        yrow[:, ic, hh * DV : (hh + 1) * DV],
                        y_ps[ic][:, :DV],
                        recip[:, 0:1],
                        None,
                        TT.mult,
                    )

            for hp in range(H // 2):
                h0 = 2 * hp
                for jc in range(JC):
                    pq = attps.tile([P, 2, SQ], F32, tag="pq")
                    for c in range(2):
                        nc.tensor.matmul(
                            pq[:, c, :],
                            kT_sb[:, jc, :],
                            qT_sb[:, h0 + c, :],
                            start=True,
                            stop=True,
                        )
                    tl = att.tile([P, 2, SQ], BF16, tag="tl")
                    bb = biasT_sb[:, jc, :].unsqueeze(1).broadcast_to([P, 2, SQ])
                    nc.vector.tensor_tensor(tl[:], pq[:], bb, TT.add)
                    tt = att.tile([P, 2, SQ], BF16, tag="tt")
                    nc.scalar.activation(
                        tt[:].rearrange("p a b -> p (a b)"),
                        tl[:].rearrange("p a b -> p (a b)"),
                        AF.Tanh,
                        scale=1.0 / SOFTCAP,
                    )
                    nc.scalar.activation(
                        pt_sb[:, jc, :, :].rearrange("p a b -> p (a b)"),
                        tt[:].rearrange("p a b -> p (a b)"),
                        AF.Exp,
                        scale=SOFTCAP,
                    )
                    attnv(0, jc)
                drain_y(0)
                for jc in range(JC):
                    attnv(1, jc)
                drain_y(1)
                # yrow [128, QC, 384] -> yT (3 feature chunks per head pair)
                for ic in range(QC):
                    for f in range(3):
                        nc.sync.dma_start_transpose(
                            yT_sb[:, hp * 3 + f, ic * P : (ic + 1) * P],
                            yrow[:, ic, f * P : (f + 1) * P],
                        )

        # =========================================================
        # Phase C: output projection
        # =========================================================
        with (
            tc.tile_pool(name="cpool", bufs=2) as cpool,
            tc.tile_pool(name="cps", bufs=2, space="PSUM") as cps,
        ):
            for ic in range(QC):
                o_ps = cps.tile([P, D], F32, tag="o")
                for fc in range(FC):
                    for n in range(D // 512):
                        nc.tensor.matmul(
                            o_ps[:, n * 512 : (n + 1) * 512],
                            yT_sb[:, fc, ic * P : (ic + 1) * P],
                            wo_sb[:, fc, n * 512 : (n + 1) * 512],
                            start=(fc == 0),
                            stop=(fc == FC - 1),
                        )
                o_sb = cpool.tile([P, D], F32, tag="osb")
                nc.vector.tensor_tensor(o_sb[:], o_ps[:], bor_sb[:], TT.add)
                nc.sync.dma_start(out[ic * P : (ic + 1) * P, :], o_sb[:])

    nc.compile()
    return nc


def _host_prep(inputs):
    f64 = np.float64
    bf = lambda a: np.ascontiguousarray(a).astype(ml_dtypes.bfloat16)
    x = np.asarray(inputs["x"], f64)
    bias = np.asarray(inputs["attention_bias"], f64)
    g1 = np.asarray(inputs["g1"], f64)
    b1 = np.asarray(inputs["b1"], f64)
    rr1 = np.asarray(inputs["rrms1"], f64)
    Wq = np.asarray(inputs["Wq"], f64)
    Wk = np.asarray(inputs["Wk"], f64)
    Wv = np.asarray(inputs["Wv"], f64)
    qg = np.asarray(inputs["qg"], f64)
    qb = np.asarray(inputs["qb"], f64)
    kg = np.asarray(inputs["kg"], f64)
    kb = np.asarray(inputs["kb"], f64)
    vg = np.asarray(inputs["vg"], f64)
    vb = np.asarray(inputs["vb"], f64)
    Wo = np.asarray(inputs["Wo"], f64)
    bo = np.asarray(inputs["bo"], f64)
    g2 = np.asarray(inputs["g2"], f64)
    b2 = np.asarray(inputs["b2"], f64)
    rr2 = np.asarray(inputs["rrms2"], f64)

    scale1 = g1 / np.sqrt(rr1 + EPS_RMS)
    Wq_e = Wq * scale1[:, None]
    Wk_e = Wk * scale1[:, None]
    Wv_e = Wv * scale1[:, None]
    bq_row = b1 @ Wq
    bk_row = b1 @ Wk
    bv_row = b1 @ Wv

    # center each output block (folds LN mean-removal into the projection)
    def center_blocks(W, brow, width):
        W = W.copy()
        brow = brow.copy()
        for s in range(0, W.shape[1], width):
            sl = slice(s, s + width)
            W[:, sl] -= W[:, sl].mean(axis=1, keepdims=True)
            brow[sl] -= brow[sl].mean()
        return W, brow

    Wq_c, bq_c = center_blocks(Wq_e, bq_row, DQ)
    Wk_c, bk_c = center_blocks(Wk_e, bk_row, DK)
    Wv_c, bv_c = center_blocks(Wv_e, bv_row, DV)
    wkv = np.concatenate([Wk_c, Wv_c], axis=1)
    bkv = np.concatenate([bk_c, bv_c])

    sc_q = DQ ** -0.5
    qg_e, qb_e = qg * sc_q, qb * sc_q

    scale2 = g2 / np.sqrt(rr2 + EPS_RMS)
    Wo_e = Wo * scale2[None, :]
    bo_e = bo * scale2 + b2 + np.tile(vb, H) @ (Wo * scale2[None, :])

    freqs = 1.0 / (ROPE_BASE ** (np.arange(HALF, dtype=f64) / HALF))
    ang = np.arange(S, dtype=f64)[:, None] * freqs[None, :]
    cos, sin = np.cos(ang), np.sin(ang)  # [S, 64]

    def fold_tiles(g, b, pos_sl, nrep):
        """P1/P2/B tiles: out = rstd*(y1*P1 + y2*P2) + B after LN+rope."""
        c, s = cos[pos_sl], sin[pos_sl]
        n = c.shape[0]
        g1h, g2h = g[:HALF], g[HALF:]
        b1h, b2h = b[:HALF], b[HALF:]
        p1 = np.concatenate([g1h * c, g1h * s], axis=1)            # [n,128]
        p2 = np.concatenate([-g2h * s, g2h * c], axis=1)
        bb = np.concatenate(
            [b1h * c - b2h * s, b1h * s + b2h * c], axis=1
        )
        if nrep > 1:
            p1 = np.tile(p1, (1, nrep))
            p2 = np.tile(p2, (1, nrep))
            bb = np.tile(bb, (1, nrep))
        return p1, p2, bb

    p1k_, p2k_, bbk_ = fold_tiles(kg, kb, slice(0, S), 1)

    rep = lambda v: np.broadcast_to(np.asarray(v)[None, :], (P, len(v)))
    shared = {
        "wkv": bf(wkv),
        "wq": bf(Wq_c),
        "wo": bf(Wo_e),
        "p1k": bf(p1k_),
        "p2k": bf(p2k_),
        "bbk": bf(bbk_),
        "bkv": bf(rep(bkv)),
        "bqr": bf(rep(bq_c)),
        "vgr": bf(rep(vg)),
        "bor": np.ascontiguousarray(rep(bo_e), dtype=np.float32),
        "ident": bf(np.eye(P)),
    }

    bias2 = bias[0, 0]  # [S_q, S_k]
    in_maps = []
    for c in range(NCORES):
        b_ = c // 4
        s0 = (c % 4) * SQ
        m = dict(shared)
        # Rotate the sequence so this core's q rows are the LAST 4 chunks
        # (attention is permutation-invariant over keys as long as k/v/bias
        # and the rope position tiles use the same order).
        pos_idx = np.concatenate(
            [np.arange(0, s0), np.arange(s0 + SQ, S), np.arange(s0, s0 + SQ)]
        )
        xb = x[b_].T  # [D, S]
        m["xT"] = bf(xb[:, pos_idx])
        bias_rows = bias2[s0 : s0 + SQ, :]  # [SQ q, S k]
        m["biasT"] = bf(bias_rows[:, pos_idx].T)  # [S k(rot), SQ q]
        p1kr, p2kr, bbkr = p1k_[pos_idx], p2k_[pos_idx], bbk_[pos_idx]
        m["p1k"], m["p2k"], m["bbk"] = bf(p1kr), bf(p2kr), bf(bbkr)
        p1q_, p2q_, bbq_ = fold_tiles(qg_e, qb_e, slice(s0, s0 + SQ), H)
        m["p1q"], m["p2q"], m["bbq"] = bf(p1q_), bf(p2q_), bf(bbq_)
        in_maps.append(m)
    return in_maps


_NC_CACHE = None


def _get_nc():
    global _NC_CACHE
    if _NC_CACHE is None:
        _NC_CACHE = build_program()
    return _NC_CACHE


def kernel(**inputs) -> np.ndarray:
    nc = _get_nc()
    in_maps = _host_prep(inputs)
    res = bass_utils.run_bass_kernel_spmd(
        nc, in_maps, core_ids=list(range(NCORES))
    )
    outs = res.results
    full = np.empty((B, S, D), np.float32)
    for c in range(NCORES):
        b_ = c // 4
        s0 = (c % 4) * SQ
        full[b_, s0 : s0 + SQ, :] = outs[c]["out"]
    return full


if __name__ == "__main__":
    nc = _get_nc()
    print("build + compile OK")


# revision 35
# speedup vs baseline: 2.1722x; 1.1071x over previous
"""Trainium2 Bass kernel for an MQA attention block (8 q-heads, shared K/V).

Sharding: 8 cores; core c -> batch b=c//4, query rows s0=(c%4)*512 .. +512,
all 8 heads.  K/V (full sequence, per batch) are computed redundantly on each
core; no cross-core communication.

v2 design (bf16 compute, fp32 PSUM):
- Projections in ROW layout: stationary = xT chunk, moving = W.  K|V fused
  into one 320-wide matmul; Q done on the same stationary for its 4 chunks.
- LayerNorm mean-removal is folded into the weights host-side (each output
  block of W is column-centered, so projections have exact zero block-mean).
  Only sum-of-squares is needed on device.
- LN gain + RoPE are folded into host-precomputed multiplier tiles P1/P2/B:
  out = rstd * (y1*P1 + y2*P2) + B  (4 DVE ops per chunk, no transposed LN).
- Softmax denominator via a ones-column appended to V (no max subtraction
  needed: logits are softcapped to +-5).
- Attention works per head-pair: logits 2 heads x 512 q -> [128,1024] tiles
  for bias-add (DVE), tanh + exp (ACT).  attn*V per head with pt stationary.
- y rows -> yT via DMA xbar transposes (overlapped with attention compute).
- Output projection from yT with Wo moving; rms2 folded into Wo host-side;
  v-LN bias folded into the output bias (softmax weights sum to 1).
"""

import os
import sys

for _p in ("/opt/trn_rl_repo",):
    if _p not in sys.path and os.path.isdir(_p):
        sys.path.insert(0, _p)

import numpy as np
import ml_dtypes
from contextlib import ExitStack

import concourse.bass as bass
import concourse.mybir as mybir
import concourse.tile as tile
from concourse import bacc
from concourse import bass_utils

F32 = mybir.dt.float32
BF16 = mybir.dt.bfloat16

# problem shapes (hardcoded per contract)
B, S, D = 2, 2048, 1536
H, DQ, DK, DV = 8, 128, 128, 192
P = 128
SQ = S // 4          # 512 query rows per core
DC = D // P          # 12 contraction chunks
JC = S // P          # 16 key/position chunks
QC = SQ // P         # 4 query-row chunks
NCORES = 8
EPS_RMS = 1e-6
EPS_LN = 1e-5
SOFTCAP = 5.0
ROPE_BASE = 8192.0
HALF = DQ // 2       # 64
VP = 194             # v row width: 192 v + 1 ones + 1 pad
HDV = H * DV         # 1536
FC = HDV // P        # 12 output-contraction chunks

TT = mybir.AluOpType
AF = mybir.ActivationFunctionType
AX = mybir.AxisListType


def build_program():
    nc = bacc.Bacc(
        "TRN2", target_bir_lowering=False, debug=False, num_devices=NCORES
    )

    def din(name, shape, dt=BF16):
        return nc.dram_tensor(name, list(shape), dt, kind="ExternalInput").ap()

    # per-core inputs -- all big tensors come pre-arranged host-side with the
    # partition dim FIRST so every DMA line is one long contiguous segment.
    xT = din("xT", (P, JC, DC, P))          # [p, key-chunk, d-chunk, i]
    biasT = din("biasT", (P, JC, SQ))       # attention bias, [key, query]
    p1q = din("p1q", (P, QC, DK))           # rope/LN fold tiles for q
    p2q = din("p2q", (P, QC, DK))
    bbq = din("bbq", (P, QC, DK))
    # shared (replicated) inputs
    wkv = din("wkv", (P, DC, DK + DV))
    wq = din("wq", (P, DC, H * DQ))
    wo = din("wo", (P, FC, D))
    p1k = din("p1k", (P, JC, DK))
    p2k = din("p2k", (P, JC, DK))
    bbk = din("bbk", (P, JC, DK))
    bkv_r = din("bkv", (P, DK + DV))        # row-replicated proj bias k|v
    bq_r = din("bqr", (P, H * DQ))          # row-replicated proj bias q
    vg_r = din("vgr", (P, DV))              # row-replicated v LN gain
    bor = din("bor", (P, D))                # row-replicated output bias
    ident = din("ident", (P, P))
    out = nc.dram_tensor("out", [SQ, D], F32, kind="ExternalOutput").ap()

    with tile.TileContext(nc) as tc, ExitStack() as ctx:
        const = ctx.enter_context(tc.tile_pool(name="const", bufs=1))
        persist = ctx.enter_context(tc.tile_pool(name="persist", bufs=1))

        ident_sb = const.tile([P, P], BF16)
        nc.sync.dma_start(ident_sb[:], ident)
        bkv_sb = const.tile([P, DK + DV], BF16)
        nc.sync.dma_start(bkv_sb[:], bkv_r)
        bq_sb = const.tile([P, H * DQ], BF16)
        nc.sync.dma_start(bq_sb[:], bq_r)
        vg_sb = const.tile([P, DV], BF16)
        nc.sync.dma_start(vg_sb[:], vg_r)
        bor_sb = const.tile([P, D], BF16)
        eps_sb = const.tile([P, 1], F32)
        nc.vector.memset(eps_sb[:], EPS_LN)

        # persistent activations
        ky = persist.tile([P, JC, DK], BF16)      # raw centered k rows
        vy = persist.tile([P, JC, DV], BF16)      # raw centered v rows
        qy = persist.tile([P, QC, H * DQ], BF16)  # raw centered q rows
        kT_sb = persist.tile([P, JC, P], BF16)    # final k, feature-major
        qT_sb = persist.tile([P, H, SQ], BF16)    # final q, feature-major
        v_sb = persist.tile([P, JC, VP], BF16)    # final v rows + ones col
        yT_sb = persist.tile([P, FC, SQ], BF16)   # attn out, feature-major
        rk = persist.tile([P, JC], F32)           # 1/std for k chunks
        rv = persist.tile([P, JC], F32)
        rq = persist.tile([P, QC * H], F32)

        # ones column + pad for v (memset once; LN writes only [:, :192])
        nc.vector.memset(v_sb[:, :, DV:], 0.0)
        for jc in range(JC):
            nc.vector.memset(v_sb[:, jc, DV : DV + 1], 1.0)

        # attention/output-phase tiles (DMAs issued inside phase A so they
        # stream behind the critical A-phase loads)
        attc = ctx.enter_context(tc.tile_pool(name="attc", bufs=1))
        biasT_sb = attc.tile([P, JC, SQ], BF16)
        pt_sb = attc.tile([P, JC, 2, SQ], BF16)
        yrow = attc.tile([P, QC, 2 * DV], BF16)

        # =========================================================
        # Phase A (merged): projections + stats + LN/RoPE-apply +
        # transposes, pipelined per chunk so DVE/ACT work runs under
        # the projection matmul stream.  q rows are chunks 4..7 (host
        # rotation) so wq has time to arrive but q is done early.
        # =========================================================
        QLO = 4
        with (
            tc.tile_pool(name="aw", bufs=1) as aw,
            tc.tile_pool(name="ax", bufs=3) as ax,
            tc.tile_pool(name="a3c", bufs=1) as a3c,
            tc.tile_pool(name="awork", bufs=3) as a3,
            tc.tile_pool(name="asq", bufs=2) as asq,
            tc.tile_pool(name="aps", bufs=2, space="PSUM") as aps,
            tc.tile_pool(name="atps", bufs=2, space="PSUM") as atps,
        ):
            wkv_sb = aw.tile([P, DC, DK + DV], BF16)
            nc.sync.dma_start(wkv_sb[:], wkv)
            wq_sb = aw.tile([P, DC, H * DQ], BF16)
            # scalar-queue order: k-apply tiles, wq, q-apply tiles, biasT
            p1k_sb = a3c.tile([P, JC, DK], BF16)
            nc.scalar.dma_start(p1k_sb[:], p1k)
            p2k_sb = a3c.tile([P, JC, DK], BF16)
            nc.scalar.dma_start(p2k_sb[:], p2k)
            bbk_sb = a3c.tile([P, JC, DK], BF16)
            nc.scalar.dma_start(bbk_sb[:], bbk)
            nc.scalar.dma_start(wq_sb[:], wq)
            p1q_sb = a3c.tile([P, QC, DK], BF16)
            nc.scalar.dma_start(p1q_sb[:], p1q)
            p2q_sb = a3c.tile([P, QC, DK], BF16)
            nc.scalar.dma_start(p2q_sb[:], p2q)
            bbq_sb = a3c.tile([P, QC, DK], BF16)
            nc.scalar.dma_start(bbq_sb[:], bbq)
            nc.scalar.dma_start(biasT_sb[:], biasT)

            def apply_k(jc):
                u = a3.tile([P, DK], BF16, tag="ku")
                y1b = ky[:, jc, 0:HALF].unsqueeze(1).broadcast_to([P, 2, HALF])
                y2b = ky[:, jc, HALF:DK].unsqueeze(1).broadcast_to([P, 2, HALF])
                nc.vector.tensor_tensor(
                    u[:].rearrange("p (t f) -> p t f", t=2),
                    p1k_sb[:, jc, :].rearrange("p (t f) -> p t f", t=2),
                    y1b,
                    TT.mult,
                )
                u2 = a3.tile([P, DK], BF16, tag="ku2")
                nc.vector.tensor_tensor(
                    u2[:].rearrange("p (t f) -> p t f", t=2),
                    p2k_sb[:, jc, :].rearrange("p (t f) -> p t f", t=2),
                    y2b,
                    TT.mult,
                )
                nc.vector.tensor_tensor(u[:], u[:], u2[:], TT.add)
                kr = a3.tile([P, DK], BF16, tag="kr")
                nc.vector.scalar_tensor_tensor(
                    out=kr[:], in0=u[:], scalar=rk[:, jc : jc + 1],
                    in1=bbk_sb[:, jc, :], op0=TT.mult, op1=TT.add,
                )
                return kr

            def apply_q(qic):
                uq = a3.tile([P, H * DQ], BF16, tag="qu")
                qv = qy[:, qic, :].rearrange("p (h t f) -> p h t f", t=2, f=HALF)
                y1b = qv[:, :, 0:1, :].broadcast_to([P, H, 2, HALF])
                y2b = qv[:, :, 1:2, :].broadcast_to([P, H, 2, HALF])
                nc.vector.tensor_tensor(
                    uq[:].rearrange("p (h t f) -> p h t f", t=2, f=HALF),
                    p1q_sb[:, qic, :]
                    .rearrange("p (t f) -> p t f", t=2)
                    .unsqueeze(1)
                    .broadcast_to([P, H, 2, HALF]),
                    y1b,
                    TT.mult,
                )
                uq2 = a3.tile([P, H * DQ], BF16, tag="qu2")
                nc.vector.tensor_tensor(
                    uq2[:].rearrange("p (h t f) -> p h t f", t=2, f=HALF),
                    p2q_sb[:, qic, :]
                    .rearrange("p (t f) -> p t f", t=2)
                    .unsqueeze(1)
                    .broadcast_to([P, H, 2, HALF]),
                    y2b,
                    TT.mult,
                )
                nc.vector.tensor_tensor(uq[:], uq[:], uq2[:], TT.add)
                rqb = (
                    rq[:, qic * H : (qic + 1) * H]
                    .unsqueeze(2)
                    .broadcast_to([P, H, DQ])
                )
                nc.vector.tensor_tensor(
                    uq[:].rearrange("p (h f) -> p h f", f=DQ),
                    uq[:].rearrange("p (h f) -> p h f", f=DQ),
                    rqb,
                    TT.mult,
                )
                qr = a3.tile([P, H * DQ], BF16, tag="qr")
                nc.vector.tensor_tensor(
                    qr[:].rearrange("p (h f) -> p h f", f=DQ),
                    uq[:].rearrange("p (h f) -> p h f", f=DQ),
                    bbq_sb[:, qic, :].unsqueeze(1).broadcast_to([P, H, DQ]),
                    TT.add,
                )
                return qr

            # transposes lag the apply chain by LAGT chunks (PE FIFO never
            # waits on the DVE apply chain)
            pend_tp = []

            def flush_tp(budget):
                for _ in range(min(budget, len(pend_tp))):
                    kind, src_t, idx = pend_tp.pop(0)
                    if kind == "k":
                        tp = atps.tile([P, P], BF16, tag="tp")
                        nc.tensor.transpose(tp[:], src_t[:], ident_sb[:])
                        nc.vector.tensor_copy(kT_sb[:, idx, :], tp[:])
                    else:
                        for h in range(H):
                            tp = atps.tile([P, P], BF16, tag="tp")
                            nc.tensor.transpose(
                                tp[:], src_t[:, h * DQ : (h + 1) * DQ],
                                ident_sb[:],
                            )
                            nc.scalar.copy(
                                qT_sb[:, h, idx * P : (idx + 1) * P], tp[:]
                            )

            for jc in range(JC):
                xt = ax.tile([P, DC, P], BF16, tag="xt")
                (nc.sync if jc % 2 == 0 else nc.scalar).dma_start(
                    xt[:], xT[:, jc, :, :]
                )
                kv_ps = aps.tile([P, DK + DV], F32, tag="kv")
                qic = jc - QLO if QLO <= jc < QLO + QC else None
                q_ps = None
                if qic is not None:
                    q_ps = aps.tile([P, H * DQ], F32, tag="q")
                for dc in range(DC):
                    nc.tensor.matmul(
                        kv_ps[:],
                        xt[:, dc, :],
                        wkv_sb[:, dc, :],
                        start=(dc == 0),
                        stop=(dc == DC - 1),
                    )
                    if qic is not None:
                        for n in range(2):
                            nc.tensor.matmul(
                                q_ps[:, n * 512 : (n + 1) * 512],
                                xt[:, dc, :],
                                wq_sb[:, dc, n * 512 : (n + 1) * 512],
                                start=(dc == 0),
                                stop=(dc == DC - 1),
                            )
                # lagged transposes keep PE busy right after this chunk's MMs
                flush_tp(2)
                # drains (bias add, cast to bf16)
                nc.vector.tensor_tensor(
                    ky[:, jc, :], kv_ps[:, :DK], bkv_sb[:, :DK], TT.add
                )
                nc.vector.tensor_tensor(
                    vy[:, jc, :], kv_ps[:, DK:], bkv_sb[:, DK:], TT.add
                )
                # stats: sumsq -> rstd (per chunk)
                sq = asq.tile([P, DK + DV], BF16, tag="sq")
                nc.scalar.activation(sq[:, :DK], ky[:, jc, :], AF.Square)
                nc.scalar.activation(sq[:, DK:], vy[:, jc, :], AF.Square)
                ss = asq.tile([P, 4], F32, tag="ss")
                nc.vector.tensor_reduce(
                    ss[:, 0:1], sq[:, :DK], axis=AX.X, op=TT.add
                )
                nc.vector.tensor_reduce(
                    ss[:, 1:2], sq[:, DK:], axis=AX.X, op=TT.add
                )
                nc.scalar.activation(
                    ss[:, 2:3], ss[:, 0:1], AF.Sqrt,
                    bias=eps_sb[:, 0:1], scale=1.0 / DK,
                )
                nc.scalar.activation(
                    ss[:, 3:4], ss[:, 1:2], AF.Sqrt,
                    bias=eps_sb[:, 0:1], scale=1.0 / DV,
                )
                nc.vector.reciprocal(rk[:, jc : jc + 1], ss[:, 2:3])
                nc.vector.reciprocal(rv[:, jc : jc + 1], ss[:, 3:4])
                # applies
                kr = apply_k(jc)
                pend_tp.append(("k", kr, jc))
                nc.vector.scalar_tensor_tensor(
                    out=v_sb[:, jc, :DV], in0=vy[:, jc, :],
                    scalar=rv[:, jc : jc + 1], in1=vg_sb[:],
                    op0=TT.mult, op1=TT.mult,
                )
                if qic is not None:
                    nc.vector.tensor_tensor(
                        qy[:, qic, :], q_ps[:], bq_sb[:], TT.add
                    )
                    qsq = asq.tile([P, H * DQ], BF16, tag="qsq")
                    nc.scalar.activation(qsq[:], qy[:, qic, :], AF.Square)
                    ssq = asq.tile([P, 2 * H], F32, tag="ssq")
                    nc.vector.tensor_reduce(
                        ssq[:, :H],
                        qsq[:].rearrange("p (h f) -> p h f", f=DQ),
                        axis=AX.X,
                        op=TT.add,
                    )
                    nc.scalar.activation(
                        ssq[:, H:], ssq[:, :H], AF.Sqrt,
                        bias=eps_sb[:, 0:1], scale=1.0 / DQ,
                    )
                    nc.vector.reciprocal(
                        rq[:, qic * H : (qic + 1) * H], ssq[:, H:]
                    )
                    qr = apply_q(qic)
                    pend_tp.append(("q", qr, qic))
            while pend_tp:
                flush_tp(len(pend_tp))

        # wo arrives during attention, reusing the space wkv/wq vacated
        attw = ctx.enter_context(tc.tile_pool(name="attw", bufs=1))
        wo_sb = attw.tile([P, FC, D], BF16)
        nc.sync.dma_start(wo_sb[:], wo)
        nc.scalar.dma_start(bor_sb[:], bor)

        # =========================================================
        # Attention (per head pair) + yT via DMA transposes
        # Software-pipelined: attn*V matmuls lag the logits stream so the
        # in-order PE queue never waits on the DVE->ACT->ACT softcap chain.
        # =========================================================
        with (
            tc.tile_pool(name="att", bufs=3) as att,
            tc.tile_pool(name="attps", bufs=2, space="PSUM") as attps,
            tc.tile_pool(name="yps", bufs=1, space="PSUM") as yps,
        ):
            # y accumulators packed 2 heads per PSUM bank
            y_ps = [
                yps.tile([P, 2, VP], F32, tag=f"y{ic}", name=f"y{ic}")
                for ic in range(QC)
            ]

            def attnv(hh, jc):
                # Both heads share one PSUM bank per ic: a single accumulation
                # group per bank (start clears the whole bank's has_written
                # bits, so only the FIRST matmul touching the bank may set it;
                # head 1's first write lands on cleared elements and stores).
                for ic in range(QC):
                    nc.tensor.matmul(
                        y_ps[ic][:, hh, :],
                        pt_sb[:, jc, hh, ic * P : (ic + 1) * P],
                        v_sb[:, jc, :],
                        start=(jc == 0 and hh == 0),
                        stop=(jc == JC - 1 and hh == 1),
                        skip_group_check=True,
                    )

            def drain_y(hh):
                for ic in range(QC):
                    recip = att.tile([P, 1], F32, tag="recip")
                    nc.vector.reciprocal(recip[:], y_ps[ic][:, hh, DV : DV + 1])
                    nc.vector.tensor_scalar(
                        yrow[:, ic, hh * DV : (hh + 1) * DV],
                        y_ps[ic][:, hh, :DV],
                        recip[:, 0:1],
                        None,
                        TT.mult,
                    )

            LAG0, LAG1 = 3, 5

            def tails(hp):
                for jc in range(JC - LAG0, JC):
                    attnv(0, jc)
                for jc in range(JC - LAG1, JC):
                    attnv(1, jc)

            def transposes(hp):
                # yrow [128, QC, 384] -> yT (3 feature chunks per head pair)
                for ic in range(QC):
                    for f in range(3):
                        nc.sync.dma_start_transpose(
                            yT_sb[:, hp * 3 + f, ic * P : (ic + 1) * P],
                            yrow[:, ic, f * P : (f + 1) * P],
                        )

            for hp in range(H // 2):
                h0 = 2 * hp
                for jc in range(JC):
                    pq = attps.tile([P, 2, SQ], F32, tag="pq")
                    for c in range(2):
                        nc.tensor.matmul(
                            pq[:, c, :],
                            kT_sb[:, jc, :],
                            qT_sb[:, h0 + c, :],
                            start=True,
                            stop=True,
                        )
                    bb = biasT_sb[:, jc, :].unsqueeze(1).broadcast_to([P, 2, SQ])
                    nc.vector.tensor_tensor(pq[:], pq[:], bb, TT.add)
                    nc.scalar.activation(
                        pq[:].rearrange("p a b -> p (a b)"),
                        pq[:].rearrange("p a b -> p (a b)"),
                        AF.Tanh,
                        scale=1.0 / SOFTCAP,
                    )
                    nc.scalar.activation(
                        pt_sb[:, jc, :, :].rearrange("p a b -> p (a b)"),
                        pq[:].rearrange("p a b -> p (a b)"),
                        AF.Exp,
                        scale=SOFTCAP,
                    )
                    # previous head pair's epilogue rides inside this hp's
                    # first units so no engine queue drains at the boundary
                    if hp > 0:
                        if jc == 0:
                            tails(hp - 1)
                        elif jc == 1:
                            drain_y(0)
                        elif jc == 2:
                            drain_y(1)
                        elif jc == 3:
                            transposes(hp - 1)
                    if jc >= LAG0:
                        attnv(0, jc - LAG0)
                    if jc >= LAG1:
                        attnv(1, jc - LAG1)
            tails(H // 2 - 1)
            drain_y(0)
            drain_y(1)
            transposes(H // 2 - 1)

        # =========================================================
        # Phase C: output projection
        # =========================================================
        with (
            tc.tile_pool(name="cpool", bufs=2) as cpool,
            tc.tile_pool(name="cps", bufs=2, space="PSUM") as cps,
        ):
            for ic in range(QC):
                o_ps = cps.tile([P, D], F32, tag="o")
                for fc in range(FC):
                    for n in range(D // 512):
                        nc.tensor.matmul(
                            o_ps[:, n * 512 : (n + 1) * 512],
                            yT_sb[:, fc, ic * P : (ic + 1) * P],
                            wo_sb[:, fc, n * 512 : (n + 1) * 512],
                            start=(fc == 0),
                            stop=(fc == FC - 1),
                        )
                o_sb = cpool.tile([P, D], F32, tag="osb")
                nc.vector.tensor_tensor(o_sb[:], o_ps[:], bor_sb[:], TT.add)
                nc.scalar.dma_start(out[ic * P : (ic + 1) * P, :], o_sb[:])

    nc.compile()
    return nc


def _host_prep(inputs):
    f64 = np.float64
    bf = lambda a: np.ascontiguousarray(a).astype(ml_dtypes.bfloat16)
    x = np.asarray(inputs["x"], f64)
    bias = np.asarray(inputs["attention_bias"], f64)
    g1 = np.asarray(inputs["g1"], f64)
    b1 = np.asarray(inputs["b1"], f64)
    rr1 = np.asarray(inputs["rrms1"], f64)
    Wq = np.asarray(inputs["Wq"], f64)
    Wk = np.asarray(inputs["Wk"], f64)
    Wv = np.asarray(inputs["Wv"], f64)
    qg = np.asarray(inputs["qg"], f64)
    qb = np.asarray(inputs["qb"], f64)
    kg = np.asarray(inputs["kg"], f64)
    kb = np.asarray(inputs["kb"], f64)
    vg = np.asarray(inputs["vg"], f64)
    vb = np.asarray(inputs["vb"], f64)
    Wo = np.asarray(inputs["Wo"], f64)
    bo = np.asarray(inputs["bo"], f64)
    g2 = np.asarray(inputs["g2"], f64)
    b2 = np.asarray(inputs["b2"], f64)
    rr2 = np.asarray(inputs["rrms2"], f64)

    scale1 = g1 / np.sqrt(rr1 + EPS_RMS)
    Wq_e = Wq * scale1[:, None]
    Wk_e = Wk * scale1[:, None]
    Wv_e = Wv * scale1[:, None]
    bq_row = b1 @ Wq
    bk_row = b1 @ Wk
    bv_row = b1 @ Wv

    # center each output block (folds LN mean-removal into the projection)
    def center_blocks(W, brow, width):
        W = W.copy()
        brow = brow.copy()
        for s in range(0, W.shape[1], width):
            sl = slice(s, s + width)
            W[:, sl] -= W[:, sl].mean(axis=1, keepdims=True)
            brow[sl] -= brow[sl].mean()
        return W, brow

    Wq_c, bq_c = center_blocks(Wq_e, bq_row, DQ)
    Wk_c, bk_c = center_blocks(Wk_e, bk_row, DK)
    Wv_c, bv_c = center_blocks(Wv_e, bv_row, DV)
    wkv = np.concatenate([Wk_c, Wv_c], axis=1)
    bkv = np.concatenate([bk_c, bv_c])

    sc_q = DQ ** -0.5
    qg_e, qb_e = qg * sc_q, qb * sc_q

    scale2 = g2 / np.sqrt(rr2 + EPS_RMS)
    Wo_e = Wo * scale2[None, :]
    bo_e = bo * scale2 + b2 + np.tile(vb, H) @ (Wo * scale2[None, :])

    freqs = 1.0 / (ROPE_BASE ** (np.arange(HALF, dtype=f64) / HALF))
    ang = np.arange(S, dtype=f64)[:, None] * freqs[None, :]
    cos, sin = np.cos(ang), np.sin(ang)  # [S, 64]

    def fold_tiles(g, b, pos_sl, nrep):
        """P1/P2/B tiles: out = rstd*(y1*P1 + y2*P2) + B after LN+rope."""
        c, s = cos[pos_sl], sin[pos_sl]
        n = c.shape[0]
        g1h, g2h = g[:HALF], g[HALF:]
        b1h, b2h = b[:HALF], b[HALF:]
        p1 = np.concatenate([g1h * c, g1h * s], axis=1)            # [n,128]
        p2 = np.concatenate([-g2h * s, g2h * c], axis=1)
        bb = np.concatenate(
            [b1h * c - b2h * s, b1h * s + b2h * c], axis=1
        )
        if nrep > 1:
            p1 = np.tile(p1, (1, nrep))
            p2 = np.tile(p2, (1, nrep))
            bb = np.tile(bb, (1, nrep))
        return p1, p2, bb

    p1k_, p2k_, bbk_ = fold_tiles(kg, kb, slice(0, S), 1)

    def parr(a):
        """[n*128, F] -> [128, n, F] so each DMA line is contiguous."""
        a = np.asarray(a)
        n = a.shape[0] // P
        return a.reshape(n, P, -1).transpose(1, 0, 2)

    rep = lambda v: np.broadcast_to(np.asarray(v)[None, :], (P, len(v)))
    shared = {
        "wkv": bf(parr(wkv)),
        "wq": bf(parr(Wq_c)),
        "wo": bf(parr(Wo_e)),
        "bkv": bf(rep(bkv)),
        "bqr": bf(rep(bq_c)),
        "vgr": bf(rep(vg)),
        "bor": bf(rep(bo_e)),
        "ident": bf(np.eye(P)),
    }

    bias2 = bias[0, 0]  # [S_q, S_k]
    in_maps = []
    for c in range(NCORES):
        b_ = c // 4
        s0 = (c % 4) * SQ
        m = dict(shared)
        # Rotate the sequence so this core's q rows are chunks 4..7
        # (attention is permutation-invariant over keys as long as k/v/bias
        # and the rope position tiles use the same order).
        others = np.concatenate([np.arange(0, s0), np.arange(s0 + SQ, S)])
        pos_idx = np.concatenate(
            [others[: 4 * P], np.arange(s0, s0 + SQ), others[4 * P :]]
        )
        xrot = x[b_].T[:, pos_idx]  # [D, S(rot)]
        # [p, jc, dc, i] with contiguous (dc, i) lines
        m["xT"] = bf(
            xrot.reshape(DC, P, JC, P).transpose(1, 2, 0, 3)
        )
        bias_rows = bias2[s0 : s0 + SQ, :]  # [SQ q, S k]
        m["biasT"] = bf(parr(bias_rows[:, pos_idx].T))
        m["p1k"], m["p2k"], m["bbk"] = (
            bf(parr(p1k_[pos_idx])),
            bf(parr(p2k_[pos_idx])),
            bf(parr(bbk_[pos_idx])),
        )
        p1q_, p2q_, bbq_ = fold_tiles(qg_e, qb_e, slice(s0, s0 + SQ), 1)
        m["p1q"], m["p2q"], m["bbq"] = (
            bf(parr(p1q_)),
            bf(parr(p2q_)),
            bf(parr(bbq_)),
        )
        in_maps.append(m)
    return in_maps


_NC_CACHE = None


def _get_nc():
    global _NC_CACHE
    if _NC_CACHE is None:
        _NC_CACHE = build_program()
    return _NC_CACHE


def kernel(**inputs) -> np.ndarray:
    nc = _get_nc()
    in_maps = _host_prep(inputs)
    res = bass_utils.run_bass_kernel_spmd(
        nc, in_maps, core_ids=list(range(NCORES))
    )
    outs = res.results
    full = np.empty((B, S, D), np.float32)
    for c in range(NCORES):
        b_ = c // 4
        s0 = (c % 4) * SQ
        full[b_, s0 : s0 + SQ, :] = outs[c]["out"]
    return full


if __name__ == "__main__":
    nc = _get_nc()
    print("build + compile OK")
